# revision 75
# baseline (speedup 1.0000x reference)
"""Local (sliding-window) attention kernel for TRN2, 8 NeuronCores.

Sharding: core c -> batch b=c//4, head-group hg=c%4 (4 heads of 16).
Each core: qkv projection for its heads (bf16 matmuls, fp32 PSUM),
banded attention, partial out-projection (its heads' columns of Wo).
Host sums 4 bf16 partials per batch and adds bo.

Per-core dataflow (all matmuls bf16, PSUM fp32):
  qkT[512,2048]  = wqk.T @ xT       (Q rows pre-scaled by 1/sqrt(hd);
                                     bias folded in via ACT evac)
  V  [2048,256]  = xT.T @ wv        (token-major; +bias via DVE into
                                     per-head 65-col layout, col 64 = 1)
  per head h (outer), key-block j (inner, 16 blocks of 128):
    S^T[k,q]   = kT_hj.T @ qT      (q-window <=384: blocks j..j+2)
    pT         = exp(S^T)  (ACT)   then band-complement triangles
                 zeroed by ONE fused DVE multiply (cols 0:128+256:384)
    ps_y[65,512] += vaug_hj.T @ pT (row 64 = softmax denominator)
  per (h, q-range g of 512) at its tail:
    yun = ps_y evac (DVE, frees PSUM bank early)
    rec = exp(-ln(denom))  (ACT, same table set as Exp)
    bc  = partition_broadcast(rec) (GPSIMD);  yT = yun * bc (DVE)
  out[2048,1024] = yT.T @ wo  (at h3 g-tails; bf16 partial to HBM)

qkT n-chunks and V tiles are emitted interleaved into the h0/h1 sweeps
so the PE stays dense while ACT works through the exp stream.
"""

import os
import sys

import numpy as np

if "/opt/trn_rl_repo" not in sys.path:
    sys.path.insert(0, "/opt/trn_rl_repo")

B, T, D = 2, 2048, 1024
H, W = 16, 256
HD = D // H          # 64
NCORES = 8
HPC = 4              # heads per core
FB = HPC * HD        # 256 f-columns per core
KC = D // 128        # 8 contraction chunks
NT = T // 128        # 16 key blocks / token tiles
NQ = T // 512        # 4 q-ranges
DELAY = 3            # S->PV software-pipeline depth

_STATE: dict = {}


def _build_module():
    import concourse.bacc as bacc
    import concourse.tile as tile
    from concourse import mybir

    # Pin the ACT table set: this kernel uses Exp, Ln, Identity (+const
    # bias), all present in natural_log_exp_and_others. Left to itself the
    # set-cover pass alternates exp_and_others <-> natural_log (2 table
    # loads x 1.28us per softmax normalize, 42us/kernel). Restricting the
    # choices to the one combined set yields a single load.
    _orig_tables = bacc.get_activation_tables

    def _one_set(arch):
        t = _orig_tables(arch)
        if "natural_log_exp_and_others" not in t:
            return t
        # Keep every key at its canonical index (the emitted
        # act_func_set_id indexes the unfiltered act_info.json list);
        # strip this kernel's functions from all other sets so the
        # set-cover pass must pick the combined one.
        mine = {
            mybir.ActivationFunctionType.Exp,
            mybir.ActivationFunctionType.Ln,
            mybir.ActivationFunctionType.Identity,
        }
        return {
            k: (v if k == "natural_log_exp_and_others" else v - mine)
            for k, v in t.items()
        }

    bacc.get_activation_tables = _one_set

    dt = mybir.dt
    AF = mybir.ActivationFunctionType
    OP = mybir.AluOpType

    nc = bacc.Bacc(
        "TRN2",
        target_bir_lowering=False,
        debug=False,
        enable_asserts=False,
        num_devices=NCORES,
    )

    f32 = dt.float32
    bf16 = dt.bfloat16
    xT_d = nc.dram_tensor("xT", [D, T], bf16, kind="ExternalInput").ap()
    # wqk columns pre-permuted host-side to [m0|m2|m1|m3] so the two
    # head01 tiles arrive in one contiguous descriptor
    wqk_d = nc.dram_tensor("wqk", [D, 2 * FB], bf16, kind="ExternalInput").ap()
    # bqk[4] ++ bvb[256] packed, fp32
    cst_d = nc.dram_tensor("cst", [128, 260], f32, kind="ExternalInput").ap()
    wv_d = nc.dram_tensor("wv", [D, FB], bf16, kind="ExternalInput").ap()
    wo_d = nc.dram_tensor("wo", [FB, D], bf16, kind="ExternalInput").ap()
    tris_d = nc.dram_tensor("tris", [128, 256], bf16, kind="ExternalInput").ap()
    out_d = nc.dram_tensor("out_p", [T, D], bf16, kind="ExternalOutput").ap()
    # SBUF column index of qkT m-tile given logical m (q01,q23,k01,k23)
    MCOL = {0: 0, 2: 1, 1: 2, 3: 3}

    with tile.TileContext(nc) as tc:
        with (
            tc.tile_pool(name="const", bufs=1) as cpool,
            tc.tile_pool(name="work", bufs=2) as wpool,
            tc.tile_pool(name="ps", bufs=3, space="PSUM") as ppool,
        ):
            # ---- persistent SBUF ----
            xT_sb = cpool.tile([128, KC, T], bf16)
            wqk_sb = cpool.tile([128, KC, 2 * FB], bf16)
            wv_sb = cpool.tile([128, KC, FB], bf16)
            wo_sb = cpool.tile([128, 2, D], bf16)
            cst_sb = cpool.tile([128, 260], f32)
            bqk_sb = cst_sb[:, 0:4]
            bvb_sb = cst_sb[:, 4:260]
            tris_sb = cpool.tile([128, 256], bf16)
            qkT_sb = cpool.tile([128, 4, T], bf16)
            vaug_sb = cpool.tile([128, NT, HPC * (HD + 1)], bf16)
            yT_sb = cpool.tile([128, 2, T], bf16)

            # ---- DMA preload (single 3D descriptors; first-matmul gate
            #      is wqk + xT chunk 0) ----
            # wqk halves (host pre-permuted to [m0|m2|m1|m3]): the first
            # descriptor alone gates the first qkT tiles.
            # wqk halves (host pre-permuted to [m0|m2|m1|m3]): the first
            # descriptor alone gates the first qkT tiles.
            for half in range(2):
                nc.sync.dma_start(
                    wqk_sb[:, :, half * 256:(half + 1) * 256],
                    wqk_d[:, half * 256:(half + 1) * 256].rearrange(
                        "(a p) c -> p a c", p=128
                    ),
                )
                if half == 0:
                    # split the first chunk so the leading a-chunks of the
                    # first qkT accumulation start on half the data
                    for ah in range(2):
                        nc.sync.dma_start(
                            xT_sb[:, ah * 4:(ah + 1) * 4, 0:512],
                            xT_d[ah * 512:(ah + 1) * 512, 0:512].rearrange(
                                "(a p) c -> p a c", p=128
                            ),
                        )
                    nc.sync.dma_start(cst_sb[:], cst_d[:])
                    nc.sync.dma_start(tris_sb[:], tris_d[:])
                    nc.sync.dma_start(
                        wv_sb[:], wv_d[:, :].rearrange("(a p) c -> p a c", p=128)
                    )
            for n in range(1, 4):
                nc.sync.dma_start(
                    xT_sb[:, :, n * 512:(n + 1) * 512],
                    xT_d[:, n * 512:(n + 1) * 512].rearrange(
                        "(a p) c -> p a c", p=128
                    ),
                )
            nc.sync.dma_start(
                wo_sb[:], wo_d[:, :].rearrange("(f p) c -> p f c", p=128)
            )
            # ones column (col 64 of each head's 65-col vaug block)
            nc.vector.memset(
                vaug_sb[:].rearrange("p t (h x) -> p t h x", x=65)[
                    :, :, :, 64:65
                ],
                1.0,
            )

            # ---- projection helpers ----
            def qkT_tile(m, n):
                ps_p = ppool.tile([128, 512], f32, tag="pj", name=f"ps_p{m}_{n}")
                for a in range(KC):
                    nc.tensor.matmul(
                        ps_p[:],
                        lhsT=wqk_sb[:, a, MCOL[m] * 128:(MCOL[m] + 1) * 128],
                        rhs=xT_sb[:, a, n * 512:(n + 1) * 512],
                        start=(a == 0),
                        stop=(a == KC - 1),
                    )
                nc.vector.tensor_scalar_add(
                    qkT_sb[:, m, n * 512:(n + 1) * 512],
                    ps_p[:],
                    bqk_sb[:, m:m + 1],
                )

            def v_tile(t):
                ps_p = ppool.tile([128, 512], f32, tag="pj", name=f"ps_v{t}")
                for a in range(KC):
                    nc.tensor.matmul(
                        ps_p[:, 0:FB],
                        lhsT=xT_sb[:, a, t * 128:(t + 1) * 128],
                        rhs=wv_sb[:, a, :],
                        start=(a == 0),
                        stop=(a == KC - 1),
                    )
                nc.vector.tensor_tensor(
                    out=vaug_sb[:, t, :].rearrange("p (h x) -> p h x", x=65)[
                        :, :, 0:64
                    ],
                    in0=ps_p[:, 0:FB].rearrange("p (h x) -> p h x", x=64),
                    in1=bvb_sb[:].rearrange("p (h x) -> p h x", x=64),
                    op=OP.add,
                )

            # ---- attention stages ----
            pT_t = {}
            ps_y = {}

            def stage_s(h, j):
                po = 64 * (h % 2)
                qwin = min(384, T - 128 * j)
                ps_s = ppool.tile([128, 384], f32, tag="ps_s", bufs=3,
                                  name=f"ps_s_{h}_{j}")
                nc.tensor.matmul(
                    ps_s[:, :qwin],
                    lhsT=qkT_sb[po:po + 64, 2 + h // 2,
                                j * 128:(j + 1) * 128],
                    rhs=qkT_sb[po:po + 64, h // 2,
                               j * 128:j * 128 + qwin],
                    start=True,
                    stop=True,
                )
                # ring covers one full prefetched sweep (16 steps x up to
                # 2 streams) plus pipeline slack
                pT = wpool.tile([128, 384], bf16, bufs=40,
                                name=f"pT_{h}_{j}", tag="pT")
                nc.scalar.activation(pT[:, :qwin], ps_s[:, :qwin], AF.Exp)
                # zero band-complement triangles: cols 0:128 (diag) and
                # 256:384 (far block) in one strided DVE op
                if qwin == 384:
                    nc.vector.tensor_tensor(
                        out=pT[:].rearrange("p (u x) -> p u x", x=128)[
                            :, 0:3:2, :
                        ],
                        in0=pT[:].rearrange("p (u x) -> p u x", x=128)[
                            :, 0:3:2, :
                        ],
                        in1=tris_sb[:].rearrange("p (u x) -> p u x", x=128),
                        op=OP.mult,
                    )
                else:
                    nc.vector.tensor_tensor(
                        out=pT[:, 0:128], in0=pT[:, 0:128],
                        in1=tris_sb[:, 0:128], op=OP.mult,
                    )
                pT_t[(h, j)] = pT

            def stage_pv(h, j):
                po = 64 * (h % 2)
                qwin = min(384, T - 128 * j)
                pT = pT_t.pop((h, j))
                for g in range((128 * j) // 512,
                               (128 * j + qwin - 1) // 512 + 1):
                    c0 = max(0, 512 * g - 128 * j)
                    c1 = min(qwin, 512 * (g + 1) - 128 * j)
                    if (h, g) not in ps_y:
                        ps_y[(h, g)] = ppool.tile(
                            [65, 512], f32, tag="ps_y", bufs=2,
                            name=f"ps_y_{h}_{g}",
                        )
                    first = (j == max(0, 4 * g - 2))
                    last = (j == 4 * g + 3)
                    d0 = 128 * j + c0 - 512 * g
                    nc.tensor.matmul(
                        ps_y[(h, g)][:, d0:d0 + (c1 - c0)],
                        lhsT=vaug_sb[:, j, h * 65:h * 65 + 65],
                        rhs=pT[:, c0:c1],
                        start=first,
                        stop=last,
                        skip_group_check=True,
                    )
                    if last:
                        normalize(h, g)

            def normalize(h, g):
                po = 64 * (h % 2)
                yps = ps_y.pop((h, g))
                ld = wpool.tile([1, 512], f32, bufs=2,
                                name=f"ld_{h}_{g}", tag="ld")
                nc.scalar.activation(ld[:], yps[64:65, :], AF.Ln)
                rec = wpool.tile([1, 512], f32, bufs=2,
                                 name=f"rec_{h}_{g}", tag="rec")
                nc.scalar.activation(rec[:], ld[:], AF.Exp, scale=-1.0)
                bc = wpool.tile([64, 512], f32, bufs=2,
                                name=f"bc_{h}_{g}", tag="bc")
                nc.gpsimd.partition_broadcast(bc[:], rec[0:1, :])
                # multiply straight out of PSUM (one PSUM operand is
                # allowed); skipping the separate evac saves a 687ns DVE
                # CAST per (h,g) at the cost of holding the ps_y bank
                # ~2 steps longer (PE has slack in these sweeps)
                nc.vector.tensor_tensor(
                    out=yT_sb[po:po + 64, h // 2, g * 512:(g + 1) * 512],
                    in0=yps[0:64, :],
                    in1=bc[:],
                    op=OP.mult,
                )
                if h == HPC - 1:
                    pending_op.extend(
                        (mt, nn)
                        for mt in range(4 * g, 4 * g + 4)
                        for nn in range(2)
                    )

            pending_op = []
            o_panel = {}

            def out_tile(mt, nn, flush=False):
                ps_o = ppool.tile(
                    [128, 512], f32, tag="pj",
                    name=f"ps_o_{mt}_{nn}",
                )
                for fc in range(2):
                    nc.tensor.matmul(
                        ps_o[:],
                        lhsT=yT_sb[:, fc, mt * 128:(mt + 1) * 128],
                        rhs=wo_sb[:, fc, nn * 512:(nn + 1) * 512],
                        start=(fc == 0),
                        stop=(fc == 1),
                    )
                if mt not in o_panel:
                    o_panel[mt] = wpool.tile(
                        [128, D], bf16, bufs=3,
                        name=f"o_{mt}", tag="o_sb",
                    )
                o_sb = o_panel[mt]
                # evac on DVE while the exp stream still owns ACT; in the
                # final flush ACT is idle, so alternate engines there
                if not flush or (mt + nn) % 2 == 0:
                    nc.vector.tensor_copy(
                        out=o_sb[:, nn * 512:(nn + 1) * 512], in_=ps_o[:]
                    )
                else:
                    nc.scalar.copy(o_sb[:, nn * 512:(nn + 1) * 512], ps_o[:])
                if nn == 1:
                    # one DMA descriptor per 128-token row panel
                    nc.sync.dma_start(
                        out_d[mt * 128:(mt + 1) * 128, :],
                        o_panel.pop(mt)[:],
                    )

            def drain_pending(k, flush=False):
                for _ in range(min(k, len(pending_op))):
                    out_tile(*pending_op.pop(0), flush=flush)

            # ---- interleaved emission schedule ----
            # pre-phase: heads01 qk for cols 0:512, V tiles 0..3
            qkT_tile(0, 0)
            qkT_tile(2, 0)
            for t in range(4):
                v_tile(t)

            # (m, n) qkT tiles interleaved into sweeps: keyed (sweep, j).
            # heads01 (m0/m2) chunks just-in-time within sweep 0 (they
            # feed BOTH the h0 and prefetched-h1 S streams); heads23
            # (m1/m3) land just ahead of the h2 S stream in sweep 1.
            inter = {
                (0, 1): [(0, 1), (2, 1)],
                (0, 5): [(0, 2), (2, 2)],
                (0, 9): [(0, 3), (2, 3)],
                (0, 12): [(1, 0)],
                (0, 14): [(3, 0)],
                (1, 0): [(1, 1)],
                (1, 1): [(3, 1)],
                (1, 4): [(1, 2)],
                (1, 5): [(3, 2)],
                (1, 8): [(1, 3)],
                (1, 9): [(3, 3)],
            }

            # Sweep s runs PV+normalize for head s while PREFETCHING the
            # S->exp->mask stream of head s+1 (pT tiles persist one full
            # sweep). ACT's exp work thus overlaps the PE-dense
            # projection phase, and sweeps 1..3 shrink to PV+norm (+
            # fillers). Head 0's own S stream runs in sweep 0 with a
            # DELAY-step pipeline; later heads consume ready pT tiles.
            for s in range(HPC):
                for j in range(NT):
                    for (m, n) in inter.get((s, j), []):
                        qkT_tile(m, n)
                    if s == 0 and 2 <= j <= 13:
                        v_tile(j + 2)
                    if s == 0:
                        stage_s(0, j)
                    if s < HPC - 1:
                        stage_s(s + 1, j)
                    if s == 0:
                        if j >= DELAY:
                            stage_pv(0, j - DELAY)
                    else:
                        stage_pv(s, j)
                    drain_pending(3 if s == HPC - 1 else 2)
                if s == 0:
                    for j in range(NT - DELAY, NT):
                        stage_pv(0, j)
                        drain_pending(2)
            drain_pending(len(pending_op), flush=True)

    nc.compile()
    from concourse.bass_interp import get_hw_module

    nc.m = get_hw_module(nc.m)
    return nc


def _shard_inputs(x, Wqkv, bqkv, Wo, bo):
    import ml_dtypes

    bf16 = ml_dtypes.bfloat16
    x = np.asarray(x, np.float32)
    Wqkv = np.asarray(Wqkv, np.float32)
    bqkv = np.asarray(bqkv, np.float32)
    Wo = np.asarray(Wo, np.float32)

    scale = 1.0 / np.sqrt(np.float32(HD))
    c_idx = np.arange(128)[:, None]
    u_idx = np.arange(128)[None, :]
    tri0 = (u_idx >= c_idx).astype(bf16)   # keys block j vs q block j
    tri1 = (u_idx < c_idx).astype(bf16)    # keys block j vs q block j+2
    tris = np.concatenate([tri0, tri1], axis=1)

    xT = [np.ascontiguousarray(x[b].T.astype(bf16)) for b in range(B)]

    in_maps = []
    for c in range(NCORES):
        b, hg = divmod(c, HPC)
        r0 = hg * FB
        Wq = Wqkv[r0:r0 + FB] * scale
        Wk = Wqkv[D + r0:D + r0 + FB]
        Wv = Wqkv[2 * D + r0:2 * D + r0 + FB]
        bq = bqkv[r0:r0 + FB] * scale
        bk = bqkv[D + r0:D + r0 + FB]
        bv = bqkv[2 * D + r0:2 * D + r0 + FB]
        # wqk col-blocks permuted to [m0|m2|m1|m3] (q01, k01, q23, k23)
        qk = np.concatenate([Wq, Wk], 0)  # rows: q(256) ++ k(256)
        wqk = np.concatenate(
            [qk[0:128], qk[256:384], qk[128:256], qk[384:512]], 0
        ).T.astype(bf16)
        bqk = np.concatenate([bq, bk]).reshape(4, 128).T  # logical m order
        cst = np.concatenate(
            [bqk, np.broadcast_to(bv[None, :], (128, FB))], axis=1
        ).astype(np.float32)
        in_maps.append({
            "xT": xT[b],
            "wqk": np.ascontiguousarray(wqk),
            "cst": np.ascontiguousarray(cst),
            "wv": np.ascontiguousarray(Wv.T.astype(bf16)),
            "wo": np.ascontiguousarray(Wo[:, r0:r0 + FB].T.astype(bf16)),
            "tris": tris,
        })
    return in_maps


def kernel(x, Wqkv, bqkv, Wo, bo):
    from concourse import bass_utils

    if "nc" not in _STATE:
        _STATE["nc"] = _build_module()
    nc = _STATE["nc"]

    in_maps = _shard_inputs(x, Wqkv, bqkv, Wo, bo)
    trace = bool(os.environ.get("TRNKERN_TRACE"))
    try:
        res = bass_utils.run_bass_kernel_spmd(
            nc,
            in_maps,
            core_ids=list(range(NCORES)),
            trace=trace,
        )
    except ModuleNotFoundError:
        # trace hook unavailable in this environment; run without trace
        res = bass_utils.run_bass_kernel_spmd(
            nc,
            in_maps,
            core_ids=list(range(NCORES)),
            trace=False,
        )
    _STATE["last"] = res

    bo = np.asarray(bo, np.float32)
    out = np.empty((B, T, D), np.float32)
    for b in range(B):
        acc = res.results[b * HPC]["out_p"].astype(np.float32)
        for hg in range(1, HPC):
            acc = acc + res.results[b * HPC + hg]["out_p"].astype(np.float32)
        out[b] = acc + bo[None, :]
    return out
